# revision 1
# baseline (speedup 1.0000x reference)
"""Bahdanau attention scores on 8 TRN2 NeuronCores (transposed energy).

Reference computation (jax, single device):
    h   = broadcast(hidden, [S, B, D])                       # hidden [B, D]
    x   = concat([h, encoder_outputs], axis=2)               # [S, B, D+E]
    en  = tanh(einsum('sbf,df->sbd', x, attn_w) + attn_b)    # [S, B, D]
    out = softmax(einsum('sbd,d->bs', en, v), axis=1)        # [B, S]

Data-parallel over batch (B=32 -> 4 per core), no collectives.  hid_proj
(= W_h @ hidden + bias, 0.07% of FLOPs) is computed on the host; host
prep is untimed.  The encoder matmul runs in bf16 (rel err ~7e-3 vs the
2e-2 gate) with x as the STATIONARY operand and W as the MOVING operand,
producing energy tiles [128 s-partitions, 1024 d-free].  With d on the
free axis:
  - the hid_proj bias is a DVE tensor_add (psum + host-broadcast hp row),
  - tanh runs on Act (no per-partition bias needed),
  - the v-weighted reduction over d is a single fused DVE
    scalar_tensor_tensor ((en bypass 1.0) * v_bc, accum over free)
    -> scores [128, 1].  (tensor_tensor_reduce computes the same thing
    but its custom-ISA lowering faults on the device runtime.)
This removes the PE v-reduction matmuls of the d-major formulation
entirely: PE work drops from 589824 to 524288 rows (218.5 us at
1 row/cycle, the bf16/fp32r roofline).  DVE (otherwise ~1% busy) absorbs
~90 us of adds/reduces, pipelined behind the PE; warm-up matmuls on a
memset scratch tile anchor the PE p-state ramp during the startup DMAs.
In the last batch's final two chunks the hp bias instead rides into PSUM
via a K=1 ones x hp bf16 matmul seeding the accumulation (+426 ns/tile
of PE) so tanh reads PSUM directly -- the backlogged DVE loses its add
pass exactly where its queue delay was stalling the PE on PSUM-pool
recycling and exposing drain latency on the final tail.

Scores for a batch accumulate as [128 s-low, 16 s-high] columns; the
softmax tail is all wide-partition ops: exp [128,16] + accum, a K=128
ones-matmul for the total, reciprocal, a K=1 ones-matmul to broadcast
rcp to all partitions, one tensor_scalar multiply, then a single DMA of
the [128, 16] tile.  The host un-permutes (transpose per batch) during
gather -- host prep/post is untimed.
"""

import numpy as np

S = 2048
B = 32
E = 1024
D = 1024
N_CORES = 8
BL = B // N_CORES
S_CHUNK = 512
N_CHUNKS = S // S_CHUNK
KT = E // 128
ST = S_CHUNK // 128          # s-tiles per chunk (4)
NCOL = N_CHUNKS * ST         # score columns per batch (16)

_COMPILED = None
LAST_RESULTS = None
PROFILE = False
TRACE_KWARGS = {}


def _build():
    import concourse.bacc as bacc
    import concourse.mybir as mybir
    from concourse.tile import TileContext
    from concourse.alu_op_type import AluOpType

    f32 = mybir.dt.float32
    f32r = mybir.dt.float32r
    bf16 = mybir.dt.bfloat16
    Tanh = mybir.ActivationFunctionType.Tanh
    Exp = mybir.ActivationFunctionType.Exp
    Copy = mybir.ActivationFunctionType.Copy
    X = mybir.AxisListType.X

    nc = bacc.Bacc("TRN2", target_bir_lowering=False, debug=False)

    encT = nc.dram_tensor("encT", [BL, N_CHUNKS, 128, KT, S_CHUNK], bf16,
                          kind="ExternalInput")
    # W moving: [e-part, k, d]
    wmT = nc.dram_tensor("wmT", [128, KT, D], bf16, kind="ExternalInput")
    # hid_proj broadcast to all 128 partitions, per local batch
    hpB = nc.dram_tensor("hpB", [BL, 128, D], f32, kind="ExternalInput")
    # v broadcast to all 128 partitions
    vB = nc.dram_tensor("vB", [128, D], f32, kind="ExternalInput")
    # hp rows on a single partition (bf16): PE bias seed for the last chunk
    hpW = nc.dram_tensor("hpW", [1, BL, D], bf16, kind="ExternalInput")
    # device-layout output: [b, s-low(partition), s-high(col)]; host transposes
    outT = nc.dram_tensor("outT", [BL, 128, NCOL], f32, kind="ExternalOutput")

    with TileContext(nc) as tc:
        with (
            tc.tile_pool(name="const", bufs=1) as constp,
            tc.tile_pool(name="mainps", bufs=3, space="PSUM") as mainps,
            tc.tile_pool(name="tailps", bufs=2, space="PSUM") as tailps,
        ):
            wm_sb = constp.tile([128, KT, D], bf16)
            hp_sb = constp.tile([128, BL, D], f32)
            v_sb = constp.tile([128, D], f32)
            x_first = constp.tile([128, KT, S_CHUNK], bf16)

            nc.sync.dma_start(out=wm_sb[:, 0, :], in_=wmT[:, 0, :])
            nc.sync.dma_start(out=x_first[:, 0:KT // 2, :],
                              in_=encT[0, 0, :, 0:KT // 2, :])
            nc.sync.dma_start(out=wm_sb[:, 1, :], in_=wmT[:, 1, :])
            nc.sync.dma_start(out=wm_sb[:, 2, :], in_=wmT[:, 2, :])
            nc.sync.dma_start(out=wm_sb[:, 3, :], in_=wmT[:, 3, :])
            nc.sync.dma_start(out=x_first[:, KT // 2:KT, :],
                              in_=encT[0, 0, :, KT // 2:KT, :])
            for k in range(4, KT):
                nc.sync.dma_start(out=wm_sb[:, k, :], in_=wmT[:, k, :])
            nc.sync.dma_start(out=hp_sb[:, 0, :], in_=hpB[0, :, :])
            nc.sync.dma_start(out=v_sb[:, :], in_=vB[:, :])
            hpw_sb = constp.tile([1, BL, D], bf16)
            nc.sync.dma_start(out=hpw_sb[:, :, :], in_=hpW[:, :, :])
            # hp for batches 1..3 is issued inside batch 0's chunk loop

            # Warm-up matmuls: anchor the PE p-state ramp during the DMA wait.
            scratch_x = constp.tile([128, 256], bf16)
            nc.vector.memset(scratch_x[:, :], 0.0)
            # ones operands for the tail's sum / broadcast micro-matmuls
            ones_col = constp.tile([128, 1], f32)
            ones_row = constp.tile([1, 128], f32)
            ones_bf = constp.tile([1, 128], bf16)
            nc.vector.memset(ones_col[:, :], 1.0)
            nc.vector.memset(ones_row[:, :], 1.0)
            nc.vector.memset(ones_bf[:, :], 1.0)
            warm_ps = mainps.tile([128, D], f32, tag="main")
            for w in range(14):
                nc.tensor.matmul(
                    warm_ps[:, 0:256], lhsT=scratch_x[:, 0:128],
                    rhs=scratch_x[:, :], start=True, stop=True,
                )

            with (
                tc.tile_pool(name="xp", bufs=3) as xp,
                tc.tile_pool(name="prep", bufs=2) as prep,
                tc.tile_pool(name="enp", bufs=4) as enp,
                tc.tile_pool(name="junkp", bufs=1) as junkp,
                tc.tile_pool(name="scorep", bufs=2) as scorep,
                tc.tile_pool(name="outp", bufs=2) as outp,
                tc.tile_pool(name="smallp", bufs=2) as smallp,
            ):
                junk = junkp.tile([128, D], f32)
                half_acc = junkp.tile([128, 3], f32)

                def tail_stage1(scores_sb, exp_sb, rowsum):
                    # exp + K=128 ones-matmul for the total sum.  For the
                    # last batch exp_sb/rowsum were built per-column already.
                    if exp_sb is None:
                        exp_sb = scorep.tile([128, NCOL], f32, tag="exp")
                        rowsum = smallp.tile([128, 1], f32, tag="rs")
                        nc.scalar.activation(
                            out=exp_sb[:, :], in_=scores_sb[:, :], func=Exp,
                            accum_out=rowsum[:, :],
                        )
                    sum_ps = tailps.tile([1, 1], f32, tag="tail")
                    nc.tensor.matmul(
                        sum_ps[:, :], lhsT=ones_col[:, :], rhs=rowsum[:, :],
                        start=True, stop=True,
                    )
                    return exp_sb, sum_ps

                def tail_stage2(b, exp_sb, sum_ps, last=False):
                    rcp_sb = smallp.tile([1, 1], f32, tag="rcp")
                    nc.vector.reciprocal(out=rcp_sb[:, :], in_=sum_ps[:, :])
                    bc_ps = tailps.tile([128, 1], f32, tag="tail")
                    nc.tensor.matmul(
                        bc_ps[:, :], lhsT=ones_row[:, :], rhs=rcp_sb[:, :],
                        start=True, stop=True,
                    )
                    rcp_bc = smallp.tile([128, 1], f32, tag="rbc")
                    nc.scalar.activation(
                        out=rcp_bc[:, :], in_=bc_ps[:, :], func=Copy,
                    )
                    o_sc = outp.tile([128, NCOL], f32)
                    if last:
                        nc.vector.tensor_scalar_mul(
                            out=o_sc[:, :], in0=exp_sb[:, :],
                            scalar1=rcp_bc[:, :],
                        )
                    else:
                        # Act-side multiply keeps the deferred tail out of
                        # the DVE queue, whose adds are the PSUM-release
                        # path at batch boundaries
                        nc.scalar.activation(
                            out=o_sc[:, :], in_=exp_sb[:, :], func=Copy,
                            scale=rcp_bc[:, :],
                        )
                    nc.sync.dma_start(out=outT[b, :, :], in_=o_sc[:, :])

                pending = None        # (b, scores_sb) awaiting tail emission
                for b in range(BL):
                    scores_sb = scorep.tile([128, NCOL], f32, tag="sc")
                    last_b = (b == BL - 1)
                    if last_b:
                        exp_inc = scorep.tile([128, NCOL], f32, tag="exp")
                    for c in range(N_CHUNKS):
                        if b == 0 and c == 0:
                            x_t = x_first
                        else:
                            x_t = xp.tile([128, KT, S_CHUNK], bf16, tag="x")
                            nc.sync.dma_start(
                                out=x_t[:, :, :], in_=encT[b, c, :, :, :],
                            )
                        if b == 0 and c >= 1:
                            # deferred hp DMAs ride behind the early x tiles
                            nc.sync.dma_start(
                                out=hp_sb[:, c, :], in_=hpB[c, :, :],
                            )
                        last_chunk = last_b and c == N_CHUNKS - 1
                        seed_chunk = last_b and c >= N_CHUNKS - 2
                        for i in range(ST):
                            col = c * ST + i
                            split_tile = last_chunk and i >= ST - 2
                            ps = mainps.tile([128, D], f32, tag="main")
                            en = enp.tile([128, D], f32, tag="en")
                            if not split_tile:
                                # k-major: one Ldweights per k, both d-halves.
                                # In the last chunk the hp bias rides into
                                # PSUM via a K=1 ones matmul (start=True
                                # seed) so tanh reads PSUM directly and the
                                # backlogged DVE loses its add pass.
                                if seed_chunk:
                                    for j in range(2):
                                        nc.tensor.matmul(
                                            ps[:, j * 512:(j + 1) * 512],
                                            lhsT=ones_bf[:, :],
                                            rhs=hpw_sb[0:1, b,
                                                       j * 512:(j + 1) * 512],
                                            start=True, stop=False,
                                        )
                                for k in range(KT):
                                    for j in range(2):
                                        nc.tensor.matmul(
                                            ps[:, j * 512:(j + 1) * 512],
                                            lhsT=x_t[:, k, i * 128:(i + 1) * 128],
                                            rhs=wm_sb[:, k, j * 512:(j + 1) * 512],
                                            start=(k == 0 and not seed_chunk),
                                            stop=(k == KT - 1),
                                        )
                                if seed_chunk:
                                    nc.scalar.activation(
                                        out=en[:, :], in_=ps[:, :], func=Tanh,
                                    )
                                else:
                                    pre = prep.tile([128, D], f32, tag="pre")
                                    nc.vector.tensor_add(
                                        out=pre[:, :], in0=ps[:, :],
                                        in1=hp_sb[:, b, :],
                                    )
                                    nc.scalar.activation(
                                        out=en[:, :], in_=pre[:, :], func=Tanh,
                                    )
                                nc.vector.scalar_tensor_tensor(
                                    out=junk[:, :], in0=en[:, :], scalar=1.0,
                                    in1=v_sb[:, :],
                                    op0=AluOpType.bypass, op1=AluOpType.mult,
                                    accum_out=scores_sb[:, col:col + 1],
                                )
                            else:
                                # last chunk: j-major so each d-half drains
                                # while the other half's matmuls run -- halves
                                # the drain latency exposed on the final tail
                                # (costs one extra Ldweights per (i, k)).
                                for j in range(2):
                                    sl = slice(j * 512, (j + 1) * 512)
                                    nc.tensor.matmul(
                                        ps[:, sl], lhsT=ones_bf[:, :],
                                        rhs=hpw_sb[0:1, b, sl],
                                        start=True, stop=False,
                                    )
                                    for k in range(KT):
                                        nc.tensor.matmul(
                                            ps[:, sl],
                                            lhsT=x_t[:, k, i * 128:(i + 1) * 128],
                                            rhs=wm_sb[:, k, sl],
                                            start=False,
                                            stop=(k == KT - 1),
                                        )
                                    nc.scalar.activation(
                                        out=en[:, sl], in_=ps[:, sl], func=Tanh,
                                    )
                                    nc.vector.scalar_tensor_tensor(
                                        out=junk[:, sl], in0=en[:, sl],
                                        scalar=1.0, in1=v_sb[:, sl],
                                        op0=AluOpType.bypass,
                                        op1=AluOpType.mult,
                                        accum_out=(
                                            scores_sb[:, col:col + 1] if j == 0
                                            else half_acc[:, 0:1]
                                        ),
                                    )
                                nc.vector.tensor_add(
                                    out=scores_sb[:, col:col + 1],
                                    in0=scores_sb[:, col:col + 1],
                                    in1=half_acc[:, 0:1],
                                )
                            if last_b:
                                # incremental exp: only the last column's exp
                                # sits on the critical tail
                                nc.scalar.activation(
                                    out=exp_inc[:, col:col + 1],
                                    in_=scores_sb[:, col:col + 1], func=Exp,
                                )
                        if pending is not None:
                            # stage the previous batch's softmax tail across
                            # this batch's chunks so its PE micro-matmuls
                            # never head-block the main stream
                            if c == 1:
                                pb, psc = pending
                                pexp, psum = tail_stage1(psc, None, None)
                                pending = (pb, psc, pexp, psum)
                            elif c == 2:
                                pb, _, pexp, psum = pending
                                tail_stage2(pb, pexp, psum, last=False)
                                pending = None
                    if not last_b:
                        pending = (b, scores_sb)
                    else:
                        rowsum = smallp.tile([128, 1], f32, tag="rs")
                        nc.vector.reduce_sum(
                            out=rowsum[:, :], in_=exp_inc[:, :], axis=X,
                        )
                        exp_sb, sum_ps = tail_stage1(None, exp_inc, rowsum)
                        tail_stage2(b, exp_sb, sum_ps, last=True)

    nc.compile()
    return nc


def kernel(hidden, encoder_outputs, attn_w, attn_b, v):
    global _COMPILED, LAST_RESULTS
    from concourse.bass_utils import run_bass_kernel_spmd

    hidden = np.ascontiguousarray(hidden, dtype=np.float32)
    encoder_outputs = np.ascontiguousarray(encoder_outputs, dtype=np.float32)
    attn_w = np.ascontiguousarray(attn_w, dtype=np.float32)
    attn_b = np.ascontiguousarray(attn_b, dtype=np.float32)
    v = np.ascontiguousarray(v, dtype=np.float32)
    assert hidden.shape == (B, D) and encoder_outputs.shape == (S, B, E)
    assert attn_w.shape == (D, E + D) and attn_b.shape == (D,) and v.shape == (D,)

    if _COMPILED is None:
        _COMPILED = _build()
    nc = _COMPILED

    import ml_dtypes

    # W moving layout [e-part, k, d]
    wmT = np.ascontiguousarray(
        attn_w[:, D:].T.reshape(KT, 128, D).transpose(1, 0, 2)
        .astype(ml_dtypes.bfloat16))
    hid_proj = hidden @ attn_w[:, :D].T + attn_b[None, :]        # [B, D]
    vB = np.ascontiguousarray(np.broadcast_to(v[None, :], (128, D)),
                          dtype=np.float32)

    in_maps = []
    for cid in range(N_CORES):
        b0 = cid * BL
        hpB = np.ascontiguousarray(
            np.broadcast_to(hid_proj[b0:b0 + BL, None, :], (BL, 128, D)),
            dtype=np.float32)
        in_maps.append({
            "encT": np.ascontiguousarray(
                encoder_outputs[:, b0:b0 + BL, :]
                .reshape(N_CHUNKS, S_CHUNK, BL, KT, 128)
                .transpose(2, 0, 4, 3, 1).astype(ml_dtypes.bfloat16)),
            "wmT": wmT,
            "hpB": hpB,
            "vB": vB,
            "hpW": np.ascontiguousarray(
                hid_proj[None, b0:b0 + BL, :].astype(ml_dtypes.bfloat16)),
        })

    res = run_bass_kernel_spmd(
        nc, in_maps, core_ids=list(range(N_CORES)),
        trace=PROFILE, **TRACE_KWARGS,
    )
    LAST_RESULTS = res
    # device emits [b, s-low(128), s-high(16)]; s = col*128 + p
    return np.concatenate(
        [res.results[c]["outT"].transpose(0, 2, 1).reshape(BL, S)
         for c in range(N_CORES)], axis=0
    ).astype(np.float32)



# revision 3
# speedup vs baseline: 1.3006x; 1.3006x over previous
"""Bahdanau attention scores on 8 TRN2 NeuronCores (fp8 DoubleRow energy).

Reference computation (jax, single device):
    h   = broadcast(hidden, [S, B, D])                       # hidden [B, D]
    x   = concat([h, encoder_outputs], axis=2)               # [S, B, D+E]
    en  = tanh(einsum('sbf,df->sbd', x, attn_w) + attn_b)    # [S, B, D]
    out = softmax(einsum('sbd,d->bs', en, v), axis=1)        # [B, S]

Data-parallel over batch (B=32 -> 4 per core), no collectives.  hid_proj
(= W_h @ hidden + bias, 0.07% of FLOPs) is computed on the host; host
prep is untimed.

The encoder matmul runs as fp8e4 (e4m3) DoubleRow-pair matmuls with a
3-term hi/lo split:  x ~ xh + xl,  W*2^6 ~ wh + wl  (each term an e4m3
round, residual re-rounded), and

    x @ W = (xh@wh + xh@wl + xl@wh) / 2^6   (+ O(eps^2) dropped term)

which lands ~1.3e-3 relative rms on the energy -- tighter than the bf16
path it replaces (2.4e-3).  W rides at scale 2^6 so its lo residual
clears the e4m3 subnormal floor; the host pre-scales hid_proj by 2^6 and
the tanh activation de-scales by 2^-6 (out = tanh(in * 2^-6)).  Each
DoubleRow matmul contracts a 256-wide e-pair block (pairs on dim 1 of
both operands) into a [128 s, 512 d] PSUM half: 24 matmuls per s-tile
replace bf16's 16 at a quarter of the per-row cost, cutting the PE
residency per s-tile from 3413 ns to 2560 ns.

With d on the free axis:
  - the hid_proj bias is a DVE tensor_add (psum + host-broadcast hp row),
  - tanh runs on Act with the 2^-6 de-scale fused in,
  - the v-weighted reduction over d is a single fused DVE
    scalar_tensor_tensor ((en bypass 1.0) * v_bc, accum over free)
    -> scores [128, 1].
Warm-up matmuls on a memset scratch tile anchor the PE p-state ramp
during the startup DMAs.  In the last tile of the last batch the matmuls
go j-major so each d-half drains while the other half computes.

Scores for a batch accumulate as [128 s-low, 16 s-high] columns; the
softmax tail is all wide-partition ops: exp [128,16] + accum, a K=128
ones-matmul for the total, reciprocal, a K=1 ones-matmul to broadcast
rcp to all partitions, one scaled-copy, then a single DMA of the
[128, 16] tile.  The host un-permutes (transpose per batch) during
gather -- host prep/post is untimed.
"""

import numpy as np

S = 2048
B = 32
E = 1024
D = 1024
N_CORES = 8
BL = B // N_CORES
S_CHUNK = 512
N_CHUNKS = S // S_CHUNK
KP = E // 256                # DoubleRow pair-groups (256 e each)
ST = S_CHUNK // 128          # s-tiles per chunk (4)
NCOL = N_CHUNKS * ST         # score columns per batch (16)
WSCALE = 64.0                # W rides at 2^6; tanh de-scales

_COMPILED = None
LAST_RESULTS = None
PROFILE = False
TRACE_KWARGS = {}


def _build():
    import concourse.bacc as bacc
    import concourse.mybir as mybir
    from concourse.tile import TileContext
    from concourse.alu_op_type import AluOpType

    f32 = mybir.dt.float32
    f8 = mybir.dt.float8e4
    bf16 = mybir.dt.bfloat16
    DR = mybir.MatmulPerfMode.DoubleRow
    Tanh = mybir.ActivationFunctionType.Tanh
    Exp = mybir.ActivationFunctionType.Exp
    Copy = mybir.ActivationFunctionType.Copy
    X = mybir.AxisListType.X

    nc = bacc.Bacc("TRN2", target_bir_lowering=False, debug=False)

    # hi/lo fp8 encoder tiles: [b, c, e%128(part), kpair, pair, s]
    encH = nc.dram_tensor("encH", [BL, N_CHUNKS, 128, KP, 2, S_CHUNK], f8,
                          kind="ExternalInput")
    encL = nc.dram_tensor("encL", [BL, N_CHUNKS, 128, KP, 2, S_CHUNK], f8,
                          kind="ExternalInput")
    # W moving: [e%128(part), kpair, pair, d], pre-scaled by 2^6
    wmH = nc.dram_tensor("wmH", [128, KP, 2, D], f8, kind="ExternalInput")
    wmL = nc.dram_tensor("wmL", [128, KP, 2, D], f8, kind="ExternalInput")
    # hid_proj * 2^6 broadcast to all 128 partitions, per local batch
    hpB = nc.dram_tensor("hpB", [BL, 128, D], f32, kind="ExternalInput")
    # v broadcast to all 128 partitions (unscaled; applies post-tanh)
    vB = nc.dram_tensor("vB", [128, D], f32, kind="ExternalInput")
    # device-layout output: [b, s-low(partition), s-high(col)]; host transposes
    outT = nc.dram_tensor("outT", [BL, 128, NCOL], f32, kind="ExternalOutput")

    with TileContext(nc) as tc:
        with (
            tc.tile_pool(name="const", bufs=1) as constp,
            tc.tile_pool(name="mainps", bufs=3, space="PSUM") as mainps,
            tc.tile_pool(name="tailps", bufs=2, space="PSUM") as tailps,
        ):
            wh_sb = constp.tile([128, KP, 2, D], f8)
            wl_sb = constp.tile([128, KP, 2, D], f8)
            hp_sb = constp.tile([128, BL, D], f32)
            v_sb = constp.tile([128, D], f32)
            xh_first = constp.tile([128, KP, 2, S_CHUNK], f8)
            xl_first = constp.tile([128, KP, 2, S_CHUNK], f8)

            # Startup DMA order tracks first-tile consumption order
            # (kp-major, terms T1=xh@wh, T2=xh@wl, T3=xl@wh inner):
            # kp0 needs wh0+xh, then wl0, then xl; later kps trail behind.
            nc.sync.dma_start(out=wh_sb[:, 0], in_=wmH[:, 0])
            nc.sync.dma_start(out=xh_first[:, :, :, :], in_=encH[0, 0])
            nc.sync.dma_start(out=wl_sb[:, 0], in_=wmL[:, 0])
            nc.sync.dma_start(out=xl_first[:, :, :, :], in_=encL[0, 0])
            for kp in range(1, KP):
                nc.sync.dma_start(out=wh_sb[:, kp], in_=wmH[:, kp])
                nc.sync.dma_start(out=wl_sb[:, kp], in_=wmL[:, kp])
            nc.sync.dma_start(out=hp_sb[:, 0, :], in_=hpB[0, :, :])
            nc.sync.dma_start(out=v_sb[:, :], in_=vB[:, :])
            # hp for batches 1..3 is issued inside batch 0's chunk loop

            # Warm-up matmuls: anchor the PE p-state ramp during the DMA wait.
            scratch_x = constp.tile([128, 256], bf16)
            nc.vector.memset(scratch_x[:, :], 0.0)
            # ones operands for the tail's sum / broadcast micro-matmuls
            ones_col = constp.tile([128, 1], f32)
            ones_row = constp.tile([1, 128], f32)
            nc.vector.memset(ones_col[:, :], 1.0)
            nc.vector.memset(ones_row[:, :], 1.0)
            warm_ps = mainps.tile([128, D], f32, tag="main")
            for w in range(14):
                nc.tensor.matmul(
                    warm_ps[:, 0:256], lhsT=scratch_x[:, 0:128],
                    rhs=scratch_x[:, :], start=True, stop=True,
                )

            with (
                tc.tile_pool(name="xhp", bufs=3) as xhp,
                tc.tile_pool(name="xlp", bufs=3) as xlp,
                tc.tile_pool(name="prep", bufs=2) as prep,
                tc.tile_pool(name="enp", bufs=4) as enp,
                tc.tile_pool(name="junkp", bufs=1) as junkp,
                tc.tile_pool(name="scorep", bufs=2) as scorep,
                tc.tile_pool(name="outp", bufs=2) as outp,
                tc.tile_pool(name="smallp", bufs=2) as smallp,
            ):
                junk = junkp.tile([128, D], f32)
                half_acc = junkp.tile([128, 3], f32)

                def tail_stage1(scores_sb, exp_sb, rowsum):
                    # exp + K=128 ones-matmul for the total sum.  For the
                    # last batch exp_sb/rowsum were built per-column already.
                    if exp_sb is None:
                        exp_sb = scorep.tile([128, NCOL], f32, tag="exp")
                        rowsum = smallp.tile([128, 1], f32, tag="rs")
                        nc.scalar.activation(
                            out=exp_sb[:, :], in_=scores_sb[:, :], func=Exp,
                            accum_out=rowsum[:, :],
                        )
                    sum_ps = tailps.tile([1, 1], f32, tag="tail")
                    nc.tensor.matmul(
                        sum_ps[:, :], lhsT=ones_col[:, :], rhs=rowsum[:, :],
                        start=True, stop=True,
                    )
                    return exp_sb, sum_ps

                def tail_stage2(b, exp_sb, sum_ps, last=False):
                    rcp_sb = smallp.tile([1, 1], f32, tag="rcp")
                    nc.vector.reciprocal(out=rcp_sb[:, :], in_=sum_ps[:, :])
                    bc_ps = tailps.tile([128, 1], f32, tag="tail")
                    nc.tensor.matmul(
                        bc_ps[:, :], lhsT=ones_row[:, :], rhs=rcp_sb[:, :],
                        start=True, stop=True,
                    )
                    rcp_bc = smallp.tile([128, 1], f32, tag="rbc")
                    nc.scalar.activation(
                        out=rcp_bc[:, :], in_=bc_ps[:, :], func=Copy,
                    )
                    o_sc = outp.tile([128, NCOL], f32)
                    if last:
                        nc.vector.tensor_scalar_mul(
                            out=o_sc[:, :], in0=exp_sb[:, :],
                            scalar1=rcp_bc[:, :],
                        )
                    else:
                        # Act-side multiply keeps the deferred tail out of
                        # the DVE queue, whose adds are the PSUM-release
                        # path at batch boundaries
                        nc.scalar.activation(
                            out=o_sc[:, :], in_=exp_sb[:, :], func=Copy,
                            scale=rcp_bc[:, :],
                        )
                    nc.sync.dma_start(out=outT[b, :, :], in_=o_sc[:, :])

                TERMS = 3

                def mm(ps_slice, x_t, w_t, kp, i, start, stop):
                    nc.tensor.matmul(
                        ps_slice,
                        lhsT=x_t[:, kp, :, i * 128:(i + 1) * 128],
                        rhs=w_t,
                        start=start, stop=stop, perf_mode=DR,
                    )

                pending = None        # (b, scores_sb) awaiting tail emission
                for b in range(BL):
                    scores_sb = scorep.tile([128, NCOL], f32, tag="sc")
                    last_b = (b == BL - 1)
                    if last_b:
                        exp_inc = scorep.tile([128, NCOL], f32, tag="exp")
                    for c in range(N_CHUNKS):
                        if b == 0 and c == 0:
                            xh_t, xl_t = xh_first, xl_first
                        else:
                            xh_t = xhp.tile([128, KP, 2, S_CHUNK], f8, tag="xh")
                            xl_t = xlp.tile([128, KP, 2, S_CHUNK], f8, tag="xl")
                            nc.sync.dma_start(out=xh_t[:, :, :, :],
                                              in_=encH[b, c])
                            nc.sync.dma_start(out=xl_t[:, :, :, :],
                                              in_=encL[b, c])
                        if b == 0 and c >= 1:
                            # deferred hp DMAs ride behind the early x tiles
                            nc.sync.dma_start(
                                out=hp_sb[:, c, :], in_=hpB[c, :, :],
                            )
                        for i in range(ST):
                            col = c * ST + i
                            split_tile = (last_b and c == N_CHUNKS - 1
                                          and i == ST - 1)
                            ps = mainps.tile([128, D], f32, tag="main")
                            en = enp.tile([128, D], f32, tag="en")
                            if not split_tile:
                                # kp-major, j inner: one stationary load per
                                # (kp, term) serves both d-halves.
                                for kp in range(KP):
                                    for t, (xs, ws) in enumerate(
                                            ((xh_t, wh_sb), (xh_t, wl_sb),
                                             (xl_t, wh_sb))):
                                        for j in range(2):
                                            mm(ps[:, j * 512:(j + 1) * 512],
                                               xs,
                                               ws[:, kp, :,
                                                  j * 512:(j + 1) * 512],
                                               kp, i,
                                               start=(kp == 0 and t == 0),
                                               stop=(kp == KP - 1
                                                     and t == TERMS - 1))
                                pre = prep.tile([128, D], f32, tag="pre")
                                nc.vector.tensor_add(
                                    out=pre[:, :], in0=ps[:, :],
                                    in1=hp_sb[:, b, :],
                                )
                                nc.scalar.activation(
                                    out=en[:, :], in_=pre[:, :], func=Tanh,
                                    scale=1.0 / WSCALE,
                                )
                                nc.vector.scalar_tensor_tensor(
                                    out=junk[:, :], in0=en[:, :], scalar=1.0,
                                    in1=v_sb[:, :],
                                    op0=AluOpType.bypass, op1=AluOpType.mult,
                                    accum_out=scores_sb[:, col:col + 1],
                                )
                            else:
                                # final tile: j-major so each d-half drains
                                # while the other half's matmuls run
                                for j in range(2):
                                    sl = slice(j * 512, (j + 1) * 512)
                                    for kp in range(KP):
                                        for t, (xs, ws) in enumerate(
                                                ((xh_t, wh_sb), (xh_t, wl_sb),
                                                 (xl_t, wh_sb))):
                                            mm(ps[:, sl], xs,
                                               ws[:, kp, :, sl], kp, i,
                                               start=(kp == 0 and t == 0),
                                               stop=(kp == KP - 1
                                                     and t == TERMS - 1))
                                    pre = prep.tile([128, D], f32, tag="pre")
                                    nc.vector.tensor_add(
                                        out=pre[:, sl], in0=ps[:, sl],
                                        in1=hp_sb[:, b, sl],
                                    )
                                    nc.scalar.activation(
                                        out=en[:, sl], in_=pre[:, sl],
                                        func=Tanh, scale=1.0 / WSCALE,
                                    )
                                    nc.vector.scalar_tensor_tensor(
                                        out=junk[:, sl], in0=en[:, sl],
                                        scalar=1.0, in1=v_sb[:, sl],
                                        op0=AluOpType.bypass,
                                        op1=AluOpType.mult,
                                        accum_out=(
                                            scores_sb[:, col:col + 1]
                                            if j == 0 else half_acc[:, 0:1]
                                        ),
                                    )
                                nc.vector.tensor_add(
                                    out=scores_sb[:, col:col + 1],
                                    in0=scores_sb[:, col:col + 1],
                                    in1=half_acc[:, 0:1],
                                )
                            if last_b:
                                # incremental exp: only the last column's exp
                                # sits on the critical tail
                                nc.scalar.activation(
                                    out=exp_inc[:, col:col + 1],
                                    in_=scores_sb[:, col:col + 1], func=Exp,
                                )
                        if pending is not None:
                            # stage the previous batch's softmax tail across
                            # this batch's chunks so its PE micro-matmuls
                            # never head-block the main stream
                            if c == 1:
                                pb, psc = pending
                                pexp, psum = tail_stage1(psc, None, None)
                                pending = (pb, psc, pexp, psum)
                            elif c == 2:
                                pb, _, pexp, psum = pending
                                tail_stage2(pb, pexp, psum, last=False)
                                pending = None
                    if not last_b:
                        pending = (b, scores_sb)
                    else:
                        rowsum = smallp.tile([128, 1], f32, tag="rs")
                        nc.vector.reduce_sum(
                            out=rowsum[:, :], in_=exp_inc[:, :], axis=X,
                        )
                        exp_sb, sum_ps = tail_stage1(None, exp_inc, rowsum)
                        tail_stage2(b, exp_sb, sum_ps, last=True)

    nc.compile()
    return nc


def kernel(hidden, encoder_outputs, attn_w, attn_b, v):
    global _COMPILED, LAST_RESULTS
    from concourse.bass_utils import run_bass_kernel_spmd

    hidden = np.ascontiguousarray(hidden, dtype=np.float32)
    encoder_outputs = np.ascontiguousarray(encoder_outputs, dtype=np.float32)
    attn_w = np.ascontiguousarray(attn_w, dtype=np.float32)
    attn_b = np.ascontiguousarray(attn_b, dtype=np.float32)
    v = np.ascontiguousarray(v, dtype=np.float32)
    assert hidden.shape == (B, D) and encoder_outputs.shape == (S, B, E)
    assert attn_w.shape == (D, E + D) and attn_b.shape == (D,) and v.shape == (D,)

    if _COMPILED is None:
        _COMPILED = _build()
    nc = _COMPILED

    import ml_dtypes

    f8 = ml_dtypes.float8_e4m3fn

    def split8(a):
        hi = a.astype(f8)
        lo = (a - hi.astype(np.float32)).astype(f8)
        return hi, lo

    # W moving layout [e%128(part), kpair, pair, d], scaled by 2^6
    wt = np.ascontiguousarray(attn_w[:, D:].T) * np.float32(WSCALE)   # [E, D]
    w_hi, w_lo = split8(wt)
    wmH = np.ascontiguousarray(
        w_hi.reshape(KP, 2, 128, D).transpose(2, 0, 1, 3))
    wmL = np.ascontiguousarray(
        w_lo.reshape(KP, 2, 128, D).transpose(2, 0, 1, 3))
    hid_proj = (hidden @ attn_w[:, :D].T + attn_b[None, :]) \
        * np.float32(WSCALE)                                          # [B, D]
    vB = np.ascontiguousarray(np.broadcast_to(v[None, :], (128, D)),
                              dtype=np.float32)

    in_maps = []
    for cid in range(N_CORES):
        b0 = cid * BL
        hpB = np.ascontiguousarray(
            np.broadcast_to(hid_proj[b0:b0 + BL, None, :], (BL, 128, D)),
            dtype=np.float32)
        enc = encoder_outputs[:, b0:b0 + BL, :]                       # [S,BL,E]
        e_hi, e_lo = split8(enc)
        # [S, BL, E] -> [BL, c, e%128, kpair, pair, s']
        def dev_layout(a):
            return np.ascontiguousarray(
                a.reshape(N_CHUNKS, S_CHUNK, BL, KP, 2, 128)
                .transpose(2, 0, 5, 3, 4, 1))
        in_maps.append({
            "encH": dev_layout(e_hi),
            "encL": dev_layout(e_lo),
            "wmH": wmH,
            "wmL": wmL,
            "hpB": hpB,
            "vB": vB,
        })

    res = run_bass_kernel_spmd(
        nc, in_maps, core_ids=list(range(N_CORES)),
        trace=PROFILE, **TRACE_KWARGS,
    )
    LAST_RESULTS = res
    # device emits [b, s-low(128), s-high(16)]; s = col*128 + p
    return np.concatenate(
        [res.results[c]["outT"].transpose(0, 2, 1).reshape(BL, S)
         for c in range(N_CORES)], axis=0
    ).astype(np.float32)
